# revision 12
# baseline (speedup 1.0000x reference)
"""Trainium2 Bass kernel for nn_PositionalEncoding (top-128 over 65536 per row).

Per core (1 of 8): one batch of x -> [256, 65536] f32.
Layout: 128 tiles of [128 partitions x 1024]; tile t holds rows 2t (parts 0-63)
and 2t+1 (parts 64-127), 1024-elem chunks per partition.

Pipeline per core:
  P1  per tile: vector.max -> top-8 values/chunk, max_index -> local idx.
  P1b sort-8 per chunk by local idx ascending (Batcher network, payload=values).
  P1c PE-transpose to row-major: BV/BI [128 tile-parts, 1024] = 2 halves x 512.
  P2  per half: 16 rounds of max8+match_replace -> t = 128th largest of BV.
  P3  per half: mask v>=t, prefix-scan ranks, clamp >128 (flag), local_scatter
      values (u16-pair trick) and global indices into rank order.
  P5  per half: pe = sin(ix*exp(c1*iye) + odd*pi/2) via ACT Exp/Sin with
      f32 range reduction; out = pe + vals.
  Flags per row: rowcnt!=128, any chunk with 8 winners (possible capture
  loss), duplicated gidx among winners -> host recomputes those rows exactly.
"""

import math

import numpy as np

_C1 = -math.log(10000.0) / 256.0
_HALF_PI = math.pi / 2.0
_INV_2PI = 1.0 / (2.0 * math.pi)
_MAGIC = 12582912.0  # 1.5 * 2^23, f32 round-to-nearest-int trick
_RED_HI = 6.28125  # 2*pi split: HI exactly representable, n*HI exact
_RED_LO = 2.0 * math.pi - 6.28125

_SORT8_STAGES = [
    [(0, 1), (2, 3), (4, 5), (6, 7)],
    [(0, 2), (1, 3), (4, 6), (5, 7)],
    [(1, 2), (5, 6)],
    [(0, 4), (1, 5), (2, 6), (3, 7)],
    [(2, 4), (3, 5)],
    [(1, 2), (3, 4), (5, 6)],
]

_NC_CACHE = {}


def _build_program(stop=9):
    import concourse.bacc as bacc
    import concourse.bass as bass
    from concourse import mybir, tile
    from concourse.masks import make_identity

    f32 = mybir.dt.float32
    u16 = mybir.dt.uint16
    i16 = mybir.dt.int16
    u8 = mybir.dt.uint8
    Alu = mybir.AluOpType
    AF = mybir.ActivationFunctionType
    AX = mybir.AxisListType
    P = 128

    nc = bacc.Bacc(
        "TRN2",
        target_bir_lowering=False,
        debug=False,
        enable_asserts=False,
        num_devices=8,
    )
    xin = nc.dram_tensor("xin", [128, 128, 1024], f32, kind="ExternalInput").ap()
    out_d = nc.dram_tensor("outv", [128, 256], f32, kind="ExternalOutput").ap()
    idx_d = nc.dram_tensor("idxs", [128, 256], u16, kind="ExternalOutput").ap()
    flags_d = nc.dram_tensor("flags", [128, 8], f32, kind="ExternalOutput").ap()

    with tile.TileContext(nc) as tc:
        with (
            tc.tile_pool(name="consts", bufs=1) as consts,
            tc.tile_pool(name="persist", bufs=1) as pp,
            tc.tile_pool(name="xload", bufs=4) as xpool,
            tc.tile_pool(name="psum", bufs=2, space="PSUM") as psum,
        ):
            ident = consts.tile([P, P], f32)
            make_identity(nc, ident[:])
            chb3 = consts.tile([P, 64, 8], f32)
            nc.gpsimd.iota(
                chb3[:],
                [[1024, 64], [0, 8]],
                channel_multiplier=0,
                allow_small_or_imprecise_dtypes=True,
            )
            chb2 = chb3[:].rearrange("p a b -> p (a b)")
            zeros = consts.tile([P, 512], f32)
            nc.gpsimd.memset(zeros[:], 0.0)

            CV = pp.tile([P, 1024], f32)
            CIu = pp.tile([P, 1024], u16)
            CIF = pp.tile([P, 1024], f32)
            BV = pp.tile([P, 1024], f32)
            BI = pp.tile([P, 1024], f32)

            # ---- Phase 1: per-chunk top-8 values + local indices ----
            for t in range(128):
                xt = xpool.tile([P, 1024], f32)
                nc.sync.dma_start(out=xt[:], in_=xin[t])
                nc.vector.max(out=CV[:, t * 8 : t * 8 + 8], in_=xt[:])
                nc.vector.max_index(
                    out=CIu[:, t * 8 : t * 8 + 8],
                    in_max=CV[:, t * 8 : t * 8 + 8],
                    in_values=xt[:],
                )
            nc.vector.tensor_copy(out=CIF[:], in_=CIu[:])

            # ---- Phase 1b: sort-8 within each chunk, key=idx asc, payload=val
            cmask = pp.tile([P, P], u8)
            tmpK = pp.tile([P, P], f32)
            tmpV = pp.tile([P, P], f32)

            def slot(ap, s):
                # [P, 128] view of slot s of every chunk (stride 8)
                return ap[:].rearrange("p (c s) -> p s c", s=8)[:, s]

            for stage in _SORT8_STAGES if stop >= 2 else []:
                for (i, j) in stage:
                    Ki, Kj = slot(CIF, i), slot(CIF, j)
                    Vi, Vj = slot(CV, i), slot(CV, j)
                    nc.vector.tensor_tensor(out=cmask[:], in0=Ki, in1=Kj, op=Alu.is_gt)
                    nc.vector.tensor_tensor(out=tmpK[:], in0=Ki, in1=Kj, op=Alu.min)
                    nc.vector.tensor_tensor(out=Kj, in0=Ki, in1=Kj, op=Alu.max)
                    nc.vector.tensor_copy(out=Ki, in_=tmpK[:])
                    nc.vector.tensor_copy(out=tmpV[:], in_=Vi)
                    nc.vector.copy_predicated(out=Vi, mask=cmask[:], data=Vj)
                    nc.vector.copy_predicated(out=Vj, mask=cmask[:], data=tmpV[:])

            # ---- Phase 1c: transpose (chunk-part, tile-free) -> (tile-part)
            # CV[p, t*8+s] -> BV[t, h*512 + c*8 + s] with p = h*64+c
            for (src, dst) in (((CV, BV), (CIF, BI)) if stop >= 3 else ()):
                dst4 = dst[:].rearrange("t (h c s) -> t h c s", h=2, s=8)
                for s in range(8):
                    pt = psum.tile([P, P], f32)
                    nc.tensor.transpose(pt[:], slot(src, s), ident[:])
                    src3 = pt[:].rearrange("t (h c) -> t h c", h=2)
                    nc.vector.tensor_copy(out=dst4[:, :, :, s], in_=src3)

            # ---- Phases 2-5 per half ----
            BVC = pp.tile([P, 512], f32)
            M8 = pp.tile([P, 8], f32)
            tvec = pp.tile([P, 2], f32)
            wm = pp.tile([P, 512], f32)
            y2 = pp.tile([P, 512], f32)
            r0 = pp.tile([P, 512], f32)
            m2 = pp.tile([P, 512], f32)
            PI2 = pp.tile([P, 1024], i16)
            gi16 = pp.tile([P, 512], i16)
            GF = pp.tile([P, 512], f32)
            GU = pp.tile([P, 512], u16)
            V128u = pp.tile([P, 256], u16)
            G128u = pp.tile([P, 128], u16)
            eqd = pp.tile([P, 511], f32)
            bothd = pp.tile([P, 511], f32)
            cnt8 = pp.tile([P, 64], f32)
            f8 = pp.tile([P, 64], f32)
            OUT = pp.tile([P, 256], f32)
            IDX = pp.tile([P, 256], u16)
            FT = pp.tile([P, 8], f32)
            IXu = pp.tile([P, 128], u16)
            IYu = pp.tile([P, 128], u16)
            ODu = pp.tile([P, 128], u16)
            IXf = pp.tile([P, 128], f32)
            IYf = pp.tile([P, 128], f32)
            ODf = pp.tile([P, 128], f32)
            Df = pp.tile([P, 128], f32)
            ARG = pp.tile([P, 128], f32)
            SN = pp.tile([P, 128], f32)
            TMP = pp.tile([P, 128], f32)
            PE = pp.tile([P, 128], f32)

            for h in (0, 1) if stop >= 4 else ():
                bvh = BV[:, h * 512 : (h + 1) * 512]
                bih = BI[:, h * 512 : (h + 1) * 512]

                # Phase 2: t = 128th largest of this half's 512 candidates
                nc.vector.max(out=M8[:], in_=bvh)
                nc.vector.match_replace(
                    out=BVC[:], in_to_replace=M8[:], in_values=bvh, imm_value=-1e30
                )
                for _ in range(15):
                    nc.vector.max(out=M8[:], in_=BVC[:])
                    nc.vector.match_replace(
                        out=BVC[:], in_to_replace=M8[:], in_values=BVC[:],
                        imm_value=-1e30,
                    )
                nc.vector.tensor_copy(out=tvec[:, h : h + 1], in_=M8[:, 7:8])
                if stop < 5:
                    continue

                # Phase 3: winner mask, ranks, scatter into rank order
                nc.vector.tensor_scalar(
                    out=wm[:], in0=bvh, scalar1=tvec[:, h : h + 1], scalar2=None,
                    op0=Alu.is_ge,
                )
                nc.vector.tensor_tensor_scan(
                    out=y2[:], data0=wm[:], data1=zeros[:], initial=0.0,
                    op0=Alu.add, op1=Alu.add,
                )
                nc.vector.tensor_tensor(out=r0[:], in0=wm[:], in1=y2[:], op=Alu.mult)
                nc.vector.tensor_scalar(
                    out=m2[:], in0=r0[:], scalar1=128.4, scalar2=None, op0=Alu.is_le
                )
                nc.vector.tensor_tensor(out=r0[:], in0=r0[:], in1=m2[:], op=Alu.mult)
                # value scatter: u16 pair indices (2r-2, 2r-1), negatives ignored
                PI2v = PI2[:].rearrange("p (c two) -> p two c", two=2)
                nc.vector.tensor_scalar(
                    out=PI2v[:, 0], in0=r0[:], scalar1=2.0, scalar2=-2.0,
                    op0=Alu.mult, op1=Alu.add,
                )
                nc.vector.tensor_scalar(
                    out=PI2v[:, 1], in0=r0[:], scalar1=2.0, scalar2=-1.0,
                    op0=Alu.mult, op1=Alu.add,
                )
                nc.gpsimd.local_scatter(
                    out_ap=V128u[:], data_ap=bvh.bitcast(u16), idxs_ap=PI2[:],
                    channels=128, num_elems=256, num_idxs=1024,
                )
                # global index scatter
                nc.vector.tensor_tensor(out=GF[:], in0=bih, in1=chb2, op=Alu.add)
                nc.vector.tensor_copy(out=GU[:], in_=GF[:])
                nc.vector.tensor_scalar(
                    out=gi16[:], in0=r0[:], scalar1=-1.0, scalar2=None, op0=Alu.add
                )
                nc.gpsimd.local_scatter(
                    out_ap=G128u[:], data_ap=GU[:], idxs_ap=gi16[:],
                    channels=128, num_elems=128, num_idxs=512,
                )
                nc.vector.tensor_copy(out=IDX[:, h * 128 : (h + 1) * 128], in_=G128u[:])

                # Flags: rowcnt, chunk-saturation, duplicate gidx among winners
                nc.vector.tensor_reduce(
                    out=FT[:, h * 4 : h * 4 + 1], in_=wm[:], axis=AX.X, op=Alu.add
                )
                wm3 = wm[:].rearrange("p (c s) -> p s c", s=8)
                nc.vector.tensor_tensor(
                    out=cnt8[:], in0=wm3[:, 0], in1=wm3[:, 1], op=Alu.add
                )
                for s in range(2, 8):
                    nc.vector.tensor_tensor(
                        out=cnt8[:], in0=cnt8[:], in1=wm3[:, s], op=Alu.add
                    )
                nc.vector.tensor_scalar(
                    out=f8[:], in0=cnt8[:], scalar1=7.5, scalar2=None, op0=Alu.is_ge
                )
                nc.vector.tensor_reduce(
                    out=FT[:, h * 4 + 1 : h * 4 + 2], in_=f8[:], axis=AX.X, op=Alu.max
                )
                nc.vector.tensor_tensor(
                    out=eqd[:], in0=GF[:, 0:511], in1=GF[:, 1:512], op=Alu.is_equal
                )
                nc.vector.tensor_tensor(
                    out=bothd[:], in0=wm[:, 0:511], in1=wm[:, 1:512], op=Alu.mult
                )
                nc.vector.tensor_tensor(
                    out=eqd[:], in0=eqd[:], in1=bothd[:], op=Alu.mult
                )
                nc.vector.tensor_reduce(
                    out=FT[:, h * 4 + 2 : h * 4 + 3], in_=eqd[:], axis=AX.X, op=Alu.max
                )
                nc.vector.memset(FT[:, h * 4 + 3 : h * 4 + 4], 0.0)
                if stop < 9:
                    continue

                # Phase 5: pe = sin(ix * exp(c1*iye) + odd*pi/2)
                nc.vector.tensor_scalar(
                    out=IYu[:], in0=G128u[:], scalar1=255, scalar2=None,
                    op0=Alu.bitwise_and,
                )
                nc.vector.tensor_scalar(
                    out=ODu[:], in0=G128u[:], scalar1=1, scalar2=None,
                    op0=Alu.bitwise_and,
                )
                nc.vector.tensor_scalar(
                    out=IXu[:], in0=G128u[:], scalar1=8, scalar2=None,
                    op0=Alu.logical_shift_right,
                )
                nc.vector.tensor_copy(out=IYf[:], in_=IYu[:])
                nc.vector.tensor_copy(out=ODf[:], in_=ODu[:])
                nc.vector.tensor_copy(out=IXf[:], in_=IXu[:])
                nc.vector.tensor_tensor(
                    out=IYf[:], in0=IYf[:], in1=ODf[:], op=Alu.subtract
                )
                nc.scalar.activation(out=Df[:], in_=IYf[:], func=AF.Exp, scale=_C1)
                nc.vector.tensor_tensor(out=ARG[:], in0=IXf[:], in1=Df[:], op=Alu.mult)
                nc.vector.tensor_scalar(
                    out=TMP[:], in0=ODf[:], scalar1=_HALF_PI, scalar2=None, op0=Alu.mult
                )
                nc.vector.tensor_tensor(out=ARG[:], in0=ARG[:], in1=TMP[:], op=Alu.add)
                nc.vector.tensor_scalar(
                    out=SN[:], in0=ARG[:], scalar1=_INV_2PI, scalar2=_MAGIC,
                    op0=Alu.mult, op1=Alu.add,
                )
                nc.vector.tensor_scalar(
                    out=SN[:], in0=SN[:], scalar1=-_MAGIC, scalar2=None, op0=Alu.add
                )
                nc.vector.tensor_scalar(
                    out=TMP[:], in0=SN[:], scalar1=_RED_HI, scalar2=None, op0=Alu.mult
                )
                nc.vector.tensor_tensor(
                    out=ARG[:], in0=ARG[:], in1=TMP[:], op=Alu.subtract
                )
                nc.vector.tensor_scalar(
                    out=TMP[:], in0=SN[:], scalar1=_RED_LO, scalar2=None, op0=Alu.mult
                )
                nc.vector.tensor_tensor(
                    out=ARG[:], in0=ARG[:], in1=TMP[:], op=Alu.subtract
                )
                nc.scalar.activation(out=PE[:], in_=ARG[:], func=AF.Sin)
                nc.vector.tensor_tensor(
                    out=OUT[:, h * 128 : (h + 1) * 128], in0=PE[:],
                    in1=V128u[:].bitcast(f32), op=Alu.add,
                )

            if stop >= 9:
                nc.sync.dma_start(out=out_d, in_=OUT[:])
                nc.sync.dma_start(out=idx_d, in_=IDX[:])
                nc.sync.dma_start(out=flags_d, in_=FT[:])
            else:
                if stop <= 2:
                    nc.sync.dma_start(out=out_d, in_=CV[:, :256])
                elif stop == 3:
                    nc.sync.dma_start(out=out_d, in_=BV[:, :256])
                elif stop == 4:
                    nc.sync.dma_start(out=out_d, in_=BVC[:, :256])
                elif stop == 5:
                    nc.sync.dma_start(
                        out=out_d[:, :128], in_=V128u[:].bitcast(f32)
                    )
                    nc.sync.dma_start(out=out_d[:, 128:], in_=zeros[:, :128])
                if stop == 5:
                    nc.sync.dma_start(out=idx_d, in_=IDX[:])
                    nc.sync.dma_start(out=flags_d, in_=FT[:])
                else:
                    junkI = pp.tile([P, 256], u16)
                    junkF = pp.tile([P, 8], f32)
                    nc.gpsimd.memset(junkI[:], 0)
                    nc.gpsimd.memset(junkF[:], 0.0)
                    nc.sync.dma_start(out=idx_d, in_=junkI[:])
                    nc.sync.dma_start(out=flags_d, in_=junkF[:])

    nc.compile()
    return nc


def get_nc(stop=9):
    if stop not in _NC_CACHE:
        _NC_CACHE[stop] = _build_program(stop)
    return _NC_CACHE[stop]


def _host_fix(row, pe_flat):
    thr = np.partition(row, -129)[-129]
    cand = np.where(row >= thr)[0]
    order = np.lexsort((cand, -row[cand]))
    sel = np.sort(cand[order[:128]])
    return pe_flat[sel] + row[sel]


def kernel(**inputs):
    x = np.ascontiguousarray(np.asarray(inputs["x"], dtype=np.float32))
    pos_enc = np.asarray(inputs["pos_enc"], dtype=np.float32)
    pe_flat = pos_enc.reshape(-1)

    from concourse import bass_utils

    nc = get_nc()
    in_maps = [{"xin": x[b].reshape(128, 128, 1024)} for b in range(8)]
    res = bass_utils.run_bass_kernel_spmd(nc, in_maps, core_ids=list(range(8)))

    out = np.empty((8, 256, 128), dtype=np.float32)
    for b in range(8):
        o = res.results[b]["outv"].reshape(256, 128)
        fl = res.results[b]["flags"].reshape(256, 4)
        bad = (fl[:, 0] != 128.0) | (fl[:, 1] > 0.0) | (fl[:, 2] > 0.0)
        out[b] = o
        if bad.any():
            xb = x[b].reshape(256, 65536)
            for r in np.nonzero(bad)[0]:
                out[b, r] = _host_fix(xb[r], pe_flat)
    return out
